# revision 21
# baseline (speedup 1.0000x reference)
"""Bloom attention Trainium2 kernel: tensor-parallel over heads on 8 cores.

Math (per head h, with slopes a_h):
  fused = X @ qkv_w.T + qkv_b ; per-head q,k,v (interleaved rows of qkv_w)
  s[q,k] = q.k/sqrt(128) + a_h*k  (causal k<=q)
  probs = softmax_k(s);  ctx = probs @ v ;  out = ctx @ dense_w.T + dense_b + residual

Design (per core c, heads 4c..4c+3), all matmul operands bf16 (f32 PSUM):
  Phase 1 (QKV):  Q.T, K.T kept SBUF-resident as qres/kres [128hd x 4h x 2048s],
                  V resident as vres [128s x 16st x 512(h,d)] -- no DRAM round trip.
                  Q pre-scaled by 1/sqrt(128) via host-scaled Wq/bq; Q/K bias via
                  activation-evac bias; V bias via DVE add of a broadcast row.
                  Heavy ph2 constants (nalb/masks) DMA'd mid-phase in chunks so
                  they never starve the X/W weight stream.
  Phase 2 (attn): scores.T chunks [128k x 512q] = K.T_tile.T @ Q.T_chunk in PSUM.
                  GPSIMD pre-adds mask_r + (-a_h*q row) into combined tiles
                  (racing ahead during phase 1); DVE: sadd = pscore + cmb/asb;
                  ACT: P~ = exp(sadd + a_h*k[per-partition bias]) -> bf16;
                  PE:  ctx~ += matmul(lhsT=vres[kt,:,hd_h], rhs=P~)
                       Z (on all 128 rows) += matmul(lhsT=ones[128,128], rhs=P~)
                  DVE: rz = recip_approx_fast(Z [128x512]); ctx = ctx~ * rz -> bf16
  AG:             AllGather ctx (bf16): full-head for heads 0-2, split in two
                  q-halves for head 3 so the tail collective is small.
  Phase 3 (dense): sweeps over all 512-seq blocks in head-group order
                  {0,1} -> {2} -> {3} (PSUM-accumulated per sweep, partials in
                  SBUF), giving each AllGather tens of us of slack; residual
                  (+bias) folded into the first partial add.
"""

import numpy as np
import ml_dtypes
import concourse.bass as bass
import concourse.bacc as bacc
import concourse.mybir as mybir
from concourse.tile import TileContext

dt = mybir.dt
AF = mybir.ActivationFunctionType
np_bf16 = ml_dtypes.bfloat16

S = 2048
H = 4096
NH = 32
HD = 128
NC = 8
HL = NH // NC            # heads per core = 4
CW = HL * HD             # per-core qkv width = 512
INV_NORM = 1.0 / np.sqrt(HD)
PASS = 1024              # seq columns per QKV pass
NT = H // 128            # 32 hid tiles
ST = S // 128            # 16 seq tiles
QC = S // 512            # 4 q chunks
MASK_VAL = -30000.0


def _alibi_slopes(n_heads):
    closest_pow2 = 2 ** int(np.floor(np.log2(n_heads)))
    base = 2.0 ** (-(2.0 ** -(np.log2(closest_pow2) - 3)))
    slopes = base ** np.arange(1, closest_pow2 + 1)
    if closest_pow2 != n_heads:
        extra_base = 2.0 ** (-(2.0 ** -(np.log2(2 * closest_pow2) - 3)))
        extra = extra_base ** np.arange(1, 2 * (n_heads - closest_pow2) + 1, 2)
        slopes = np.concatenate([slopes, extra])
    return slopes.astype(np.float32)


def build_nc():
    nc = bacc.Bacc("TRN2", target_bir_lowering=False)

    bf16, f32 = dt.bfloat16, dt.float32
    # ---- I/O -------------------------------------------------------------
    XT = nc.dram_tensor("XT", [H, S], bf16, kind="ExternalInput")
    WqT = nc.dram_tensor("WqT", [H, CW], bf16, kind="ExternalInput")
    WkT = nc.dram_tensor("WkT", [H, CW], bf16, kind="ExternalInput")
    WvT = nc.dram_tensor("WvT", [H, CW], bf16, kind="ExternalInput")
    qb_cols = nc.dram_tensor("qb_cols", [128, 2 * HL], f32, kind="ExternalInput")
    bvr = nc.dram_tensor("bvr", [128, CW], f32, kind="ExternalInput")
    nalb = nc.dram_tensor("nalb", [128, HL * S], f32, kind="ExternalInput")
    alibi_cols = nc.dram_tensor("alibi_cols", [128, HL * ST], f32, kind="ExternalInput")
    masks = nc.dram_tensor("masks", [128, 4 * 512], f32, kind="ExternalInput")
    ones128 = nc.dram_tensor("ones128", [128, 128], bf16, kind="ExternalInput")
    WdT = nc.dram_tensor("WdT", [H, CW], bf16, kind="ExternalInput")
    rescomb = nc.dram_tensor("rescomb", [S, CW], f32, kind="ExternalInput")
    out = nc.dram_tensor("out", [S, CW], f32, kind="ExternalOutput")

    with TileContext(nc) as tc:
        with tc.tile_pool(name="dram", bufs=1, space="DRAM") as dram, \
             tc.tile_pool(name="const", bufs=1) as cpool:
            # context tiles: full-head AG for heads 0-2, two q-halves for
            # head 3 so the tail collective is small
            ctxLs = [dram.tile([HD, S], bf16, name=f"ctxL{g}")
                     for g in range(HL - 1)]
            ctxFs = [dram.tile([NC * HD, S], bf16, addr_space="Shared",
                               name=f"ctxF{g}") for g in range(HL - 1)]
            ctxL3 = [dram.tile([HD, S // 2], bf16, name=f"ctxL3_{j}")
                     for j in range(2)]
            ctxF3 = [dram.tile([NC * HD, S // 2], bf16, addr_space="Shared",
                               name=f"ctxF3_{j}") for j in range(2)]

            # small constants resident for the whole kernel (~7 KB/partition)
            c_ones = cpool.tile([128, 128], bf16)
            c_qb = cpool.tile([128, 2 * HL], f32)
            c_acol = cpool.tile([128, HL * ST], f32)
            nc.gpsimd.dma_start(c_ones[:], ones128[:])
            nc.gpsimd.dma_start(c_qb[:], qb_cols[:])
            nc.gpsimd.dma_start(c_acol[:], alibi_cols[:])

            # dense-weight pool opened first so the attention-lifetime pool
            # below can be released before phase 3 (LIFO pool order); its
            # tiles are only allocated after phase 1 ends.
            wdp_cm = tc.tile_pool(name="wdp", bufs=1)
            wdp = wdp_cm.__enter__()

            # ---- attention-lifetime residents (freed before dense) ------
            p12_cm = tc.tile_pool(name="p12", bufs=1)
            p12 = p12_cm.__enter__()
            qres = p12.tile([128, HL, S], bf16, name="qres")
            kres = p12.tile([128, HL, S], bf16, name="kres")
            vres = p12.tile([128, ST, CW], bf16, name="vres")
            bvb = p12.tile([128, CW], f32, name="bvb")
            c_nal = p12.tile([128, HL * S], f32, name="c_nal")
            c_masks = p12.tile([128, 4 * 512], f32, name="c_masks")

            # ---- Phase 1: QKV projection --------------------------------
            with nc.named_scope("qkv"), \
                 tc.tile_pool(name="p1_sb", bufs=1) as sb1, \
                 tc.tile_pool(name="p1_ps", bufs=8, space="PSUM") as ps1:
                def qk_block(p, which, W, xts, load_x=False):
                    accs = [ps1.tile([128, 512], f32, tag="pacc",
                                     name=f"acc_{p}_{which}_{i}")
                            for i in range(2 * HL)]
                    for ht in range(NT):
                        if load_x:
                            xt = sb1.tile([128, PASS], bf16, tag="xt", bufs=34,
                                          name=f"xt_{p}_{ht}")
                            nc.sync.dma_start(
                                xt[:], XT[ht * 128:(ht + 1) * 128,
                                          p * PASS:(p + 1) * PASS])
                            xts.append(xt)
                        wt = sb1.tile([128, CW], bf16, tag="wt", bufs=8,
                                      name=f"w_{p}_{which}_{ht}")
                        nc.sync.dma_start(
                            wt[:], W[ht * 128:(ht + 1) * 128, :])
                        for h in range(HL):
                            for q2 in range(PASS // 512):
                                nc.tensor.matmul(
                                    accs[h * 2 + q2][:],
                                    wt[:, h * 128:(h + 1) * 128],
                                    xts[ht][:, q2 * 512:(q2 + 1) * 512],
                                    start=(ht == 0), stop=(ht == NT - 1))
                    dst = qres if which == 0 else kres
                    for h in range(HL):
                        for q2 in range(PASS // 512):
                            nc.scalar.activation(
                                dst[:, h, p * PASS + q2 * 512:
                                    p * PASS + (q2 + 1) * 512],
                                accs[h * 2 + q2][:], AF.Identity,
                                bias=c_qb[:, which * HL + h:which * HL + h + 1])

                def v_block(p, xts):
                    vaccs = [ps1.tile([128, CW], f32, tag="pacc",
                                      name=f"vacc_{p}_{i}")
                             for i in range(PASS // 128)]
                    for ht in range(NT):
                        wt = sb1.tile([128, CW], bf16, tag="wt", bufs=8,
                                      name=f"wv_{p}_{ht}")
                        nc.sync.dma_start(wt[:], WvT[ht * 128:(ht + 1) * 128, :])
                        for st in range(PASS // 128):
                            nc.tensor.matmul(
                                vaccs[st][:],
                                xts[ht][:, st * 128:(st + 1) * 128],
                                wt[:],
                                start=(ht == 0), stop=(ht == NT - 1))
                    for st in range(PASS // 128):
                        nc.vector.tensor_add(
                            vres[:, p * (PASS // 128) + st, :],
                            vaccs[st][:], bvb[:])

                for p in range(S // PASS):
                    xts = []
                    qk_block(p, 0, WqT, xts, load_x=True)
                    if p == 0:
                        # heavy ph2 constants: load after the first block's
                        # X/W DMA burst, in bounded chunks (sync queue order
                        # delays each trigger past another block of wt loads)
                        nc.sync.dma_start(c_nal[:, :HL * S // 2],
                                          nalb[:, :HL * S // 2])
                        nc.sync.dma_start(bvb[:], bvr[:])
                        nc.sync.dma_start(c_masks[:], masks[:])
                        v_block(p, xts)
                        nc.sync.dma_start(c_nal[:, HL * S // 2:],
                                          nalb[:, HL * S // 2:])
                        qk_block(p, 1, WkT, xts)
                    else:
                        qk_block(p, 1, WkT, xts)
                        v_block(p, xts)

            # ---- Phases 2+3 share dense weights + g0 ctx prefetch --------
            wd = wdp.tile([128, NT, CW], bf16, tag="wd", bufs=1, name="wd")

            # ---- Phase 2: attention per head ----------------------------
            with nc.named_scope("attn"), \
                 tc.tile_pool(name="p2_sb", bufs=1) as sb2, \
                 tc.tile_pool(name="p2_ps", bufs=1, space="PSUM") as ps2:
                for g in range(8):
                    nc.sync.dma_start(
                        wd[:, g * 4:(g + 1) * 4, :],
                        WdT.rearrange("(t p) e -> p t e", p=128)[
                            :, g * 4:(g + 1) * 4, :])
                cmbs = {}

                def emit_cmb(h, qc):
                    asb_s = c_nal[:, h * S + qc * 512:h * S + (qc + 1) * 512]
                    tiles = []
                    for r4 in range(4):
                        t = sb2.tile([128, 512], f32, tag="cmb", bufs=20,
                                     name=f"cmb_{h}_{qc}_{r4}")
                        nc.gpsimd.tensor_add(
                            t[:], c_masks[:, r4 * 512:(r4 + 1) * 512], asb_s)
                        tiles.append(t)
                    cmbs[(h, qc)] = tiles

                for qc in range(QC):
                    emit_cmb(0, qc)
                for h in range(HL):
                    for qc in range(QC):
                        asb = c_nal[:, h * S + qc * 512:h * S + (qc + 1) * 512]
                        pctx = ps2.tile([128, 512], f32, tag="pctx", bufs=2,
                                        name=f"pctx_{h}_{qc}")
                        pz = ps2.tile([128, 512], f32, tag="pz", bufs=2,
                                      name=f"pz_{h}_{qc}")
                        nkt = 4 * qc + 4
                        for kt in range(nkt):
                            r = kt - 4 * qc
                            c0 = 128 * r if r > 0 else 0
                            pscore = ps2.tile([128, 512], f32, tag="ps", bufs=4,
                                              name=f"ps_{h}_{qc}_{kt}")
                            nc.tensor.matmul(
                                pscore[:, c0:],
                                kres[:, h, kt * 128:(kt + 1) * 128],
                                qres[:, h, qc * 512 + c0:(qc + 1) * 512],
                                start=True, stop=True)
                            addend = cmbs[(h, qc)][r] if r >= 0 else asb
                            sadd = sb2.tile([128, 512], f32, tag="sadd", bufs=6,
                                            name=f"sadd_{h}_{qc}_{kt}")
                            nc.vector.tensor_add(sadd[:, c0:], pscore[:, c0:],
                                                 addend[:, c0:])
                            pt = sb2.tile([128, 512], bf16, tag="pt", bufs=6,
                                          name=f"pt_{h}_{qc}_{kt}")
                            nc.scalar.activation(
                                pt[:, c0:], sadd[:, c0:], AF.Exp,
                                bias=c_acol[:, h * ST + kt:h * ST + kt + 1])
                            nc.tensor.matmul(
                                pctx[:, c0:], vres[:, kt, h * 128:(h + 1) * 128],
                                pt[:, c0:],
                                start=(kt == 0), stop=(kt == nkt - 1))
                            nc.tensor.matmul(pz[:, c0:], c_ones[:], pt[:, c0:],
                                             start=(kt == 0), stop=(kt == nkt - 1))
                        if h + 1 < HL:
                            emit_cmb(h + 1, qc)
                        rz = sb2.tile([128, 512], f32, tag="rz", bufs=2,
                                      name=f"rz_{h}_{qc}")
                        nc.vector.reciprocal_approx_fast(rz[:], pz[:])
                        cx = sb2.tile([128, 512], bf16, tag="cx", bufs=3,
                                      name=f"cx_{h}_{qc}")
                        nc.vector.tensor_mul(cx[:], pctx[:], rz[:])
                        if h < HL - 1:
                            nc.sync.dma_start(
                                ctxLs[h][:, qc * 512:(qc + 1) * 512], cx[:])
                        else:
                            nc.sync.dma_start(
                                ctxL3[qc // 2][:, (qc % 2) * 512:
                                               (qc % 2) * 512 + 512], cx[:])
                            if qc == 1:
                                with nc.named_scope("ag3a"):
                                    nc.gpsimd.collective_compute(
                                        "AllGather", mybir.AluOpType.bypass,
                                        replica_groups=[list(range(NC))],
                                        ins=[ctxL3[0].opt()],
                                        outs=[ctxF3[0].opt()])
                    if h >= 1:
                        g_prev = h - 1
                        with nc.named_scope(f"ag{g_prev}"):
                            nc.gpsimd.collective_compute(
                                "AllGather", mybir.AluOpType.bypass,
                                replica_groups=[list(range(NC))],
                                ins=[ctxLs[g_prev].opt()],
                                outs=[ctxFs[g_prev].opt()])
                    if h == HL - 1:
                        with nc.named_scope("ag3b"):
                            nc.gpsimd.collective_compute(
                                "AllGather", mybir.AluOpType.bypass,
                                replica_groups=[list(range(NC))],
                                ins=[ctxL3[1].opt()], outs=[ctxF3[1].opt()])
            p12_cm.__exit__(None, None, None)

            # ---- Phase 3: dense + residual ------------------------------
            with nc.named_scope("dense"), \
                 tc.tile_pool(name="p3_sb", bufs=1) as sb3, \
                 tc.tile_pool(name="p3_ps", bufs=8, space="PSUM") as ps3:
                # head-groups 0-2 first (no dependency on the tail AG of
                # head 3), partials to SBUF; then a g=3 sweep finishes.
                accs = [sb3.tile([128, CW], f32, tag="dacc", bufs=16,
                                 name=f"dacc_{i}") for i in range(ST)]
                for sb_i in range(S // 512):
                    pos = [ps3.tile([128, CW], f32, tag="po",
                                    name=f"po_{sb_i}_{i}") for i in range(4)]
                    for g in range(2):
                        for r in range(NC):
                            ht = 4 * r + g
                            cxt = sb3.tile([128, 512], bf16, tag="cxt",
                                           bufs=8, name=f"cxt_{g}_{sb_i}_{r}")
                            nc.sync.dma_start(
                                cxt[:], ctxFs[g][
                                    r * 128:(r + 1) * 128,
                                    sb_i * 512:(sb_i + 1) * 512])
                            for st4 in range(4):
                                nc.tensor.matmul(
                                    pos[st4][:],
                                    cxt[:, st4 * 128:(st4 + 1) * 128],
                                    wd[:, ht, :],
                                    start=(g == 0 and r == 0),
                                    stop=(g == 1 and r == NC - 1))
                    for st4 in range(4):
                        st = sb_i * 4 + st4
                        res = sb3.tile([128, CW], f32, tag="res",
                                       bufs=4, name=f"res_{st}")
                        nc.sync.dma_start(
                            res[:], rescomb[st * 128:(st + 1) * 128, :])
                        nc.vector.tensor_add(accs[st][:], pos[st4][:], res[:])
                for sb_i in range(S // 512):
                    pos = [ps3.tile([128, CW], f32, tag="po",
                                    name=f"po2_{sb_i}_{i}") for i in range(4)]
                    for r in range(NC):
                        ht = 4 * r + 2
                        cxt = sb3.tile([128, 512], bf16, tag="cxt",
                                       bufs=8, name=f"cxt2_{sb_i}_{r}")
                        nc.sync.dma_start(
                            cxt[:], ctxFs[2][
                                r * 128:(r + 1) * 128,
                                sb_i * 512:(sb_i + 1) * 512])
                        for st4 in range(4):
                            nc.tensor.matmul(
                                pos[st4][:],
                                cxt[:, st4 * 128:(st4 + 1) * 128],
                                wd[:, ht, :],
                                start=(r == 0), stop=(r == NC - 1))
                    for st4 in range(4):
                        st = sb_i * 4 + st4
                        nc.vector.tensor_add(accs[st][:], pos[st4][:],
                                             accs[st][:])
                for sb_i in range(S // 512):
                    pos = [ps3.tile([128, CW], f32, tag="po",
                                    name=f"po3_{sb_i}_{i}") for i in range(4)]
                    for r in range(NC):
                        ht = 4 * r + HL - 1
                        cxt = sb3.tile([128, 512], bf16, tag="cxt",
                                       bufs=8, name=f"cxt3_{sb_i}_{r}")
                        nc.sync.dma_start(
                            cxt[:], ctxF3[sb_i // 2][
                                r * 128:(r + 1) * 128,
                                (sb_i % 2) * 512:(sb_i % 2) * 512 + 512])
                        for st4 in range(4):
                            nc.tensor.matmul(
                                pos[st4][:],
                                cxt[:, st4 * 128:(st4 + 1) * 128],
                                wd[:, ht, :],
                                start=(r == 0), stop=(r == NC - 1))
                    for st4 in range(4):
                        st = sb_i * 4 + st4
                        ob = sb3.tile([128, CW], f32, tag="ob", bufs=4,
                                      name=f"ob_{st}")
                        nc.vector.tensor_add(ob[:], pos[st4][:], accs[st][:])
                        nc.sync.dma_start(
                            out[st * 128:(st + 1) * 128, :], ob[:])
            wdp_cm.__exit__(None, None, None)

    nc.compile()
    return nc


def prep_inputs(hidden_states, residual, alibi, attention_mask,
                qkv_w, qkv_b, dense_w, dense_b):
    """Full inputs -> list of 8 per-core input maps."""
    del attention_mask  # deterministic causal mask is baked into the kernel
    X = np.asarray(hidden_states, np.float32).reshape(S, H)
    XTfull = np.ascontiguousarray(X.T).astype(np_bf16)      # [H, S]
    res = np.asarray(residual, np.float32).reshape(S, H)
    alibi = np.asarray(alibi, np.float32).reshape(NH, S)    # slopes*pos
    qkv_w = np.asarray(qkv_w, np.float32).reshape(NH, 3, HD, H)
    qkv_b = np.asarray(qkv_b, np.float32).reshape(NH, 3, HD)
    dense_w = np.asarray(dense_w, np.float32)               # [H, H]
    dense_b = np.asarray(dense_b, np.float32)

    # diag masks by r = kt - 4*qc : allow kp <= qf - 128*r
    kp = np.arange(128)[:, None]
    qf = np.arange(512)[None, :]
    m4 = np.stack([np.where(kp <= qf - 128 * r, 0.0, MASK_VAL)
                   for r in range(4)], 0).astype(np.float32)       # [4,128,512]
    masks = np.ascontiguousarray(
        m4.transpose(1, 0, 2).reshape(128, 4 * 512))

    in_maps = []
    for c in range(NC):
        hs = list(range(c * HL, (c + 1) * HL))
        WqT = np.ascontiguousarray(
            (qkv_w[hs, 0] * INV_NORM).reshape(CW, H).T).astype(np_bf16)
        WkT = np.ascontiguousarray(qkv_w[hs, 1].reshape(CW, H).T).astype(np_bf16)
        WvT = np.ascontiguousarray(qkv_w[hs, 2].reshape(CW, H).T).astype(np_bf16)
        bq = (qkv_b[hs, 0] * INV_NORM).reshape(HL, HD)      # [4,128]
        bk = qkv_b[hs, 1].reshape(HL, HD)
        qb_cols = np.ascontiguousarray(
            np.concatenate([bq, bk], 0).T)                  # [128, 8]
        bvr = np.ascontiguousarray(np.tile(
            qkv_b[hs, 2].reshape(1, CW).astype(np.float32), (128, 1)))
        al = alibi[hs]                                      # [4, S]
        nalb_a = np.ascontiguousarray(np.tile(
            (-al).reshape(1, HL * S).astype(np.float32), (128, 1)))
        acol = np.ascontiguousarray(
            al.reshape(HL, ST, 128).transpose(2, 0, 1).reshape(128, HL * ST))
        cols = slice(c * CW, (c + 1) * CW)
        WdT = np.ascontiguousarray(dense_w[cols, :].T).astype(np_bf16)
        rescomb = np.ascontiguousarray(res[:, cols] + dense_b[None, cols])
        in_maps.append({
            "XT": XTfull, "WqT": WqT, "WkT": WkT, "WvT": WvT,
            "qb_cols": qb_cols, "bvr": bvr,
            "nalb": nalb_a, "alibi_cols": acol, "masks": masks,
            "ones128": np.ones((128, 128), dtype=np_bf16),
            "WdT": WdT, "rescomb": rescomb,
        })
    return in_maps


def assemble(results):
    return np.concatenate([r["out"] for r in results], axis=1).reshape(1, S, H)


# ---------------------------------------------------------------------------
# Harness entry point
# ---------------------------------------------------------------------------
from concourse.bass_utils import run_bass_kernel_spmd

_NC_CACHE = {}


def _get_nc():
    if "nc" not in _NC_CACHE:
        _NC_CACHE["nc"] = build_nc()
    return _NC_CACHE["nc"]


def kernel(**inputs):
    """Full (unsharded) Bloom-attention block on 8 NeuronCores.

    Shards tensor-parallel over heads (4 heads/core): per-core QKV
    projection + causal alibi attention, AllGather of the context, and a
    column-sharded dense projection with residual. Returns [1, 2048, 4096]
    float32.
    """
    nc = _get_nc()
    in_maps = prep_inputs(**inputs)
    res = run_bass_kernel_spmd(nc, in_maps, core_ids=list(range(NC)))
    return assemble(res.results).astype(np.float32)


def _kernel_profiled(**inputs):
    """kernel() + NTFF profiling; returns (output, hw_exec_time_ns)."""
    import sys as _sys
    import types as _types
    import concourse.bass_utils as _bu
    _bu.upload_artifacts = lambda tmpdir: "local://" + tmpdir
    if "antenv.axon_hooks" not in _sys.modules:
        try:
            from trn_agent_boot.trn_boot import _ntff_profile_via_ctypes
            _hook = _ntff_profile_via_ctypes("/opt/axon/libaxon_pjrt.so")
            _mod = _types.ModuleType("antenv.axon_hooks")
            _mod.get_axon_ntff_profile_hook = lambda: _hook
            _mod.set_axon_ntff_profile_hook = lambda h: None
            _sys.modules["antenv.axon_hooks"] = _mod
        except Exception:
            pass
    nc = _get_nc()
    in_maps = prep_inputs(**inputs)
    res = run_bass_kernel_spmd(nc, in_maps, core_ids=list(range(NC)),
                               trace=True)
    return assemble(res.results).astype(np.float32), res.exec_time_ns


# revision 23
# speedup vs baseline: 1.0932x; 1.0932x over previous
"""Bloom attention Trainium2 kernel: tensor-parallel over heads on 8 cores.

Math (per head h, with slopes a_h):
  fused = X @ qkv_w.T + qkv_b ; per-head q,k,v (interleaved rows of qkv_w)
  s[q,k] = q.k/sqrt(128) + a_h*k  (causal k<=q)
  probs = softmax_k(s);  ctx = probs @ v ;  out = ctx @ dense_w.T + dense_b + residual

Design (per core c, heads 4c..4c+3), all matmul operands bf16 (f32 PSUM):
  Phase 1 (QKV):  Q.T, K.T kept SBUF-resident as qres/kres [128hd x 4h x 2048s],
                  V resident as vres [128s x 16st x 512(h,d)] -- no DRAM round trip.
                  Q pre-scaled by 1/sqrt(128) via host-scaled Wq/bq; Q/K bias via
                  activation-evac bias; V bias via DVE add of a broadcast row.
                  Heavy ph2 constants (nalb/masks) DMA'd mid-phase in chunks so
                  they never starve the X/W weight stream.
  Phase 2 (attn): scores.T chunks [128k x 512q] = K.T_tile.T @ Q.T_chunk in PSUM.
                  GPSIMD pre-adds mask_r + (-a_h*q row) into combined tiles
                  (racing ahead during phase 1); DVE: sadd = pscore + cmb/asb;
                  ACT: P~ = exp(sadd + a_h*k[per-partition bias]) -> bf16;
                  PE:  ctx~ += matmul(lhsT=vres[kt,:,hd_h], rhs=P~)
                       Z (on all 128 rows) += matmul(lhsT=ones[128,128], rhs=P~)
                  DVE: rz = recip_approx_fast(Z [128x512]); ctx = ctx~ * rz -> bf16
  AG:             AllGather ctx (bf16): full-head for heads 0-2, split in two
                  q-halves for head 3 so the tail collective is small.
  Phase 3 (dense): sweeps over all 512-seq blocks in head-group order
                  {0,1} -> {2} -> {3} (PSUM-accumulated per sweep, partials in
                  SBUF), giving each AllGather tens of us of slack; residual
                  (+bias) folded into the first partial add.
"""

import numpy as np
import ml_dtypes
import concourse.bass as bass
import concourse.bacc as bacc
import concourse.mybir as mybir
from concourse.tile import TileContext

dt = mybir.dt
AF = mybir.ActivationFunctionType
np_bf16 = ml_dtypes.bfloat16

S = 2048
H = 4096
NH = 32
HD = 128
NC = 8
HL = NH // NC            # heads per core = 4
CW = HL * HD             # per-core qkv width = 512
INV_NORM = 1.0 / np.sqrt(HD)
PASS = 1024              # seq columns per QKV pass
NT = H // 128            # 32 hid tiles
ST = S // 128            # 16 seq tiles
QC = S // 512            # 4 q chunks
MASK_VAL = -30000.0


def _alibi_slopes(n_heads):
    closest_pow2 = 2 ** int(np.floor(np.log2(n_heads)))
    base = 2.0 ** (-(2.0 ** -(np.log2(closest_pow2) - 3)))
    slopes = base ** np.arange(1, closest_pow2 + 1)
    if closest_pow2 != n_heads:
        extra_base = 2.0 ** (-(2.0 ** -(np.log2(2 * closest_pow2) - 3)))
        extra = extra_base ** np.arange(1, 2 * (n_heads - closest_pow2) + 1, 2)
        slopes = np.concatenate([slopes, extra])
    return slopes.astype(np.float32)


def build_nc():
    nc = bacc.Bacc("TRN2", target_bir_lowering=False)

    bf16, f32 = dt.bfloat16, dt.float32
    # ---- I/O -------------------------------------------------------------
    XT = nc.dram_tensor("XT", [H, S], bf16, kind="ExternalInput")
    WqT = nc.dram_tensor("WqT", [H, CW], bf16, kind="ExternalInput")
    WkT = nc.dram_tensor("WkT", [H, CW], bf16, kind="ExternalInput")
    WvT = nc.dram_tensor("WvT", [H, CW], bf16, kind="ExternalInput")
    qb_cols = nc.dram_tensor("qb_cols", [128, 2 * HL], f32, kind="ExternalInput")
    nalb = nc.dram_tensor("nalb", [128, HL * S], f32, kind="ExternalInput")
    alibi_cols = nc.dram_tensor("alibi_cols", [128, HL * ST], f32, kind="ExternalInput")
    masks = nc.dram_tensor("masks", [128, 4 * 512], f32, kind="ExternalInput")
    ones128 = nc.dram_tensor("ones128", [128, 128], bf16, kind="ExternalInput")
    WdT = nc.dram_tensor("WdT", [H, CW], bf16, kind="ExternalInput")
    rescomb = nc.dram_tensor("rescomb", [S, CW], f32, kind="ExternalInput")
    out = nc.dram_tensor("out", [S, CW], f32, kind="ExternalOutput")

    with TileContext(nc) as tc:
        with tc.tile_pool(name="dram", bufs=1, space="DRAM") as dram, \
             tc.tile_pool(name="const", bufs=1) as cpool:
            # context tiles: full-head AG for heads 0-2, two q-halves for
            # head 3 so the tail collective is small
            ctxLs = [dram.tile([HD, S], bf16, name=f"ctxL{g}")
                     for g in range(HL - 1)]
            ctxFs = [dram.tile([NC * HD, S], bf16, addr_space="Shared",
                               name=f"ctxF{g}") for g in range(HL - 1)]
            ctxL3 = [dram.tile([HD, S // 2], bf16, name=f"ctxL3_{j}")
                     for j in range(2)]
            ctxF3 = [dram.tile([NC * HD, S // 2], bf16, addr_space="Shared",
                               name=f"ctxF3_{j}") for j in range(2)]

            # small constants resident for the whole kernel (~7 KB/partition)
            c_ones = cpool.tile([128, 128], bf16)
            c_qb = cpool.tile([128, 2 * HL], f32)
            c_acol = cpool.tile([128, HL * ST], f32)
            nc.gpsimd.dma_start(c_ones[:], ones128[:])
            nc.gpsimd.dma_start(c_qb[:], qb_cols[:])
            nc.gpsimd.dma_start(c_acol[:], alibi_cols[:])

            # dense-weight pool opened first so the attention-lifetime pool
            # below can be released before phase 3 (LIFO pool order); its
            # tiles are only allocated after phase 1 ends.
            wdp_cm = tc.tile_pool(name="wdp", bufs=1)
            wdp = wdp_cm.__enter__()

            # ---- attention-lifetime residents (freed before dense) ------
            p12_cm = tc.tile_pool(name="p12", bufs=1)
            p12 = p12_cm.__enter__()
            qres = p12.tile([128, HL, S], bf16, name="qres")
            kres = p12.tile([128, HL, S], bf16, name="kres")
            vres = p12.tile([128, ST, CW], bf16, name="vres")
            c_nal = p12.tile([128, HL * S], f32, name="c_nal")
            c_masks = p12.tile([128, 4 * 512], f32, name="c_masks")

            # ---- Phase 1: QKV projection --------------------------------
            with nc.named_scope("qkv"), \
                 tc.tile_pool(name="p1_sb", bufs=1) as sb1, \
                 tc.tile_pool(name="p1_ps", bufs=8, space="PSUM") as ps1:
                def qk_block(p, which, W, xts, load_x=False):
                    accs = [ps1.tile([128, 512], f32, tag="pacc",
                                     name=f"acc_{p}_{which}_{i}")
                            for i in range(2 * HL)]
                    for ht in range(NT):
                        if load_x:
                            xt = sb1.tile([128, PASS], bf16, tag="xt", bufs=34,
                                          name=f"xt_{p}_{ht}")
                            nc.sync.dma_start(
                                xt[:], XT[ht * 128:(ht + 1) * 128,
                                          p * PASS:(p + 1) * PASS])
                            xts.append(xt)
                        wt = sb1.tile([128, CW], bf16, tag="wt", bufs=8,
                                      name=f"w_{p}_{which}_{ht}")
                        nc.sync.dma_start(
                            wt[:], W[ht * 128:(ht + 1) * 128, :])
                        for h in range(HL):
                            for q2 in range(PASS // 512):
                                nc.tensor.matmul(
                                    accs[h * 2 + q2][:],
                                    wt[:, h * 128:(h + 1) * 128],
                                    xts[ht][:, q2 * 512:(q2 + 1) * 512],
                                    start=(ht == 0), stop=(ht == NT - 1))
                    dst = qres if which == 0 else kres
                    for h in range(HL):
                        for q2 in range(PASS // 512):
                            nc.scalar.activation(
                                dst[:, h, p * PASS + q2 * 512:
                                    p * PASS + (q2 + 1) * 512],
                                accs[h * 2 + q2][:], AF.Identity,
                                bias=c_qb[:, which * HL + h:which * HL + h + 1])

                def v_block(p, xts):
                    vaccs = [ps1.tile([128, CW], f32, tag="pacc",
                                      name=f"vacc_{p}_{i}")
                             for i in range(PASS // 128)]
                    for ht in range(NT):
                        wt = sb1.tile([128, CW], bf16, tag="wt", bufs=8,
                                      name=f"wv_{p}_{ht}")
                        nc.sync.dma_start(wt[:], WvT[ht * 128:(ht + 1) * 128, :])
                        for st in range(PASS // 128):
                            nc.tensor.matmul(
                                vaccs[st][:],
                                xts[ht][:, st * 128:(st + 1) * 128],
                                wt[:],
                                start=(ht == 0), stop=(ht == NT - 1))
                    for st in range(PASS // 128):
                        # v-bias folded into rescomb on the host (bv*Z/Z = bv
                        # flows through the dense layer as a constant per out
                        # column); evac is a pure copy, split ACT/DVE so the
                        # tail chain at the phase boundary halves.
                        dstv = vres[:, p * (PASS // 128) + st, :]
                        if st % 2 == 0:
                            nc.scalar.activation(dstv, vaccs[st][:],
                                                 AF.Identity)
                        else:
                            nc.vector.tensor_copy(dstv, vaccs[st][:])

                for p in range(S // PASS):
                    xts = []
                    qk_block(p, 0, WqT, xts, load_x=True)
                    if p == 0:
                        # heavy ph2 constants: load after the first block's
                        # X/W DMA burst, in bounded chunks (sync queue order
                        # delays each trigger past another block of wt loads)
                        nc.sync.dma_start(c_nal[:, :HL * S // 2],
                                          nalb[:, :HL * S // 2])
                        nc.sync.dma_start(c_masks[:], masks[:])
                        v_block(p, xts)
                        nc.sync.dma_start(c_nal[:, HL * S // 2:],
                                          nalb[:, HL * S // 2:])
                        qk_block(p, 1, WkT, xts)
                    else:
                        qk_block(p, 1, WkT, xts)
                        v_block(p, xts)

            # ---- Phases 2+3 share dense weights + g0 ctx prefetch --------
            wd = wdp.tile([128, NT, CW], bf16, tag="wd", bufs=1, name="wd")

            # ---- Phase 2: attention per head ----------------------------
            with nc.named_scope("attn"), \
                 tc.tile_pool(name="p2_sb", bufs=1) as sb2, \
                 tc.tile_pool(name="p2_ps", bufs=1, space="PSUM") as ps2:
                for g in range(8):
                    nc.sync.dma_start(
                        wd[:, g * 4:(g + 1) * 4, :],
                        WdT.rearrange("(t p) e -> p t e", p=128)[
                            :, g * 4:(g + 1) * 4, :])
                cmbs = {}

                def emit_cmb(h, qc):
                    asb_s = c_nal[:, h * S + qc * 512:h * S + (qc + 1) * 512]
                    tiles = []
                    for r4 in range(4):
                        t = sb2.tile([128, 512], f32, tag="cmb", bufs=20,
                                     name=f"cmb_{h}_{qc}_{r4}")
                        nc.gpsimd.tensor_add(
                            t[:], c_masks[:, r4 * 512:(r4 + 1) * 512], asb_s)
                        tiles.append(t)
                    cmbs[(h, qc)] = tiles

                for qc in range(QC):
                    emit_cmb(0, qc)
                for h in range(HL):
                    for qc in range(QC):
                        asb = c_nal[:, h * S + qc * 512:h * S + (qc + 1) * 512]
                        pctx = ps2.tile([128, 512], f32, tag="pctx", bufs=2,
                                        name=f"pctx_{h}_{qc}")
                        pz = ps2.tile([128, 512], f32, tag="pz", bufs=2,
                                      name=f"pz_{h}_{qc}")
                        nkt = 4 * qc + 4
                        for kt in range(nkt):
                            r = kt - 4 * qc
                            pscore = ps2.tile([128, 512], f32, tag="ps", bufs=4,
                                              name=f"ps_{h}_{qc}_{kt}")
                            nc.tensor.matmul(
                                pscore[:],
                                kres[:, h, kt * 128:(kt + 1) * 128],
                                qres[:, h, qc * 512:(qc + 1) * 512],
                                start=True, stop=True)
                            addend = cmbs[(h, qc)][r] if r >= 0 else asb
                            sadd = sb2.tile([128, 512], f32, tag="sadd", bufs=6,
                                            name=f"sadd_{h}_{qc}_{kt}")
                            nc.vector.tensor_add(sadd[:], pscore[:], addend[:])
                            pt = sb2.tile([128, 512], bf16, tag="pt", bufs=6,
                                          name=f"pt_{h}_{qc}_{kt}")
                            nc.scalar.activation(
                                pt[:], sadd[:], AF.Exp,
                                bias=c_acol[:, h * ST + kt:h * ST + kt + 1])
                            nc.tensor.matmul(
                                pctx[:], vres[:, kt, h * 128:(h + 1) * 128],
                                pt[:],
                                start=(kt == 0), stop=(kt == nkt - 1))
                            nc.tensor.matmul(pz[:], c_ones[:], pt[:],
                                             start=(kt == 0), stop=(kt == nkt - 1))
                        if h + 1 < HL:
                            emit_cmb(h + 1, qc)
                        rz = sb2.tile([128, 512], f32, tag="rz", bufs=2,
                                      name=f"rz_{h}_{qc}")
                        nc.vector.reciprocal_approx_fast(rz[:], pz[:])
                        cx = sb2.tile([128, 512], bf16, tag="cx", bufs=3,
                                      name=f"cx_{h}_{qc}")
                        nc.vector.tensor_mul(cx[:], pctx[:], rz[:])
                        if h < HL - 1:
                            nc.sync.dma_start(
                                ctxLs[h][:, qc * 512:(qc + 1) * 512], cx[:])
                        else:
                            nc.sync.dma_start(
                                ctxL3[qc // 2][:, (qc % 2) * 512:
                                               (qc % 2) * 512 + 512], cx[:])
                            if qc == 1:
                                with nc.named_scope("ag3a"):
                                    nc.gpsimd.collective_compute(
                                        "AllGather", mybir.AluOpType.bypass,
                                        replica_groups=[list(range(NC))],
                                        ins=[ctxL3[0].opt()],
                                        outs=[ctxF3[0].opt()])
                    if h < HL - 1:
                        with nc.named_scope(f"ag{h}"):
                            nc.gpsimd.collective_compute(
                                "AllGather", mybir.AluOpType.bypass,
                                replica_groups=[list(range(NC))],
                                ins=[ctxLs[h].opt()], outs=[ctxFs[h].opt()])
                    else:
                        with nc.named_scope("ag3b"):
                            nc.gpsimd.collective_compute(
                                "AllGather", mybir.AluOpType.bypass,
                                replica_groups=[list(range(NC))],
                                ins=[ctxL3[1].opt()], outs=[ctxF3[1].opt()])
            p12_cm.__exit__(None, None, None)

            # ---- Phase 3: dense + residual ------------------------------
            with nc.named_scope("dense"), \
                 tc.tile_pool(name="p3_sb", bufs=1) as sb3, \
                 tc.tile_pool(name="p3_ps", bufs=8, space="PSUM") as ps3:
                # head-groups 0-2 first (no dependency on the tail AG of
                # head 3), partials to SBUF; then a g=3 sweep finishes.
                accs = [sb3.tile([128, CW], f32, tag="dacc", bufs=16,
                                 name=f"dacc_{i}") for i in range(ST)]
                for sb_i in range(S // 512):
                    pos = [ps3.tile([128, CW], f32, tag="po",
                                    name=f"po_{sb_i}_{i}") for i in range(4)]
                    for g in range(2):
                        for r in range(NC):
                            ht = 4 * r + g
                            cxt = sb3.tile([128, 512], bf16, tag="cxt",
                                           bufs=8, name=f"cxt_{g}_{sb_i}_{r}")
                            nc.sync.dma_start(
                                cxt[:], ctxFs[g][
                                    r * 128:(r + 1) * 128,
                                    sb_i * 512:(sb_i + 1) * 512])
                            for st4 in range(4):
                                nc.tensor.matmul(
                                    pos[st4][:],
                                    cxt[:, st4 * 128:(st4 + 1) * 128],
                                    wd[:, ht, :],
                                    start=(g == 0 and r == 0),
                                    stop=(g == 1 and r == NC - 1))
                    for st4 in range(4):
                        st = sb_i * 4 + st4
                        res = sb3.tile([128, CW], f32, tag="res",
                                       bufs=4, name=f"res_{st}")
                        nc.sync.dma_start(
                            res[:], rescomb[st * 128:(st + 1) * 128, :])
                        nc.vector.tensor_add(accs[st][:], pos[st4][:], res[:])
                for sb_i in range(S // 512):
                    pos = [ps3.tile([128, CW], f32, tag="po",
                                    name=f"po2_{sb_i}_{i}") for i in range(4)]
                    for r in range(NC):
                        ht = 4 * r + 2
                        cxt = sb3.tile([128, 512], bf16, tag="cxt",
                                       bufs=8, name=f"cxt2_{sb_i}_{r}")
                        nc.sync.dma_start(
                            cxt[:], ctxFs[2][
                                r * 128:(r + 1) * 128,
                                sb_i * 512:(sb_i + 1) * 512])
                        for st4 in range(4):
                            nc.tensor.matmul(
                                pos[st4][:],
                                cxt[:, st4 * 128:(st4 + 1) * 128],
                                wd[:, ht, :],
                                start=(r == 0), stop=(r == NC - 1))
                    for st4 in range(4):
                        st = sb_i * 4 + st4
                        nc.vector.tensor_add(accs[st][:], pos[st4][:],
                                             accs[st][:])
                for sb_i in range(S // 512):
                    pos = [ps3.tile([128, CW], f32, tag="po",
                                    name=f"po3_{sb_i}_{i}") for i in range(4)]
                    for r in range(NC):
                        ht = 4 * r + HL - 1
                        cxt = sb3.tile([128, 512], bf16, tag="cxt",
                                       bufs=8, name=f"cxt3_{sb_i}_{r}")
                        nc.sync.dma_start(
                            cxt[:], ctxF3[sb_i // 2][
                                r * 128:(r + 1) * 128,
                                (sb_i % 2) * 512:(sb_i % 2) * 512 + 512])
                        for st4 in range(4):
                            nc.tensor.matmul(
                                pos[st4][:],
                                cxt[:, st4 * 128:(st4 + 1) * 128],
                                wd[:, ht, :],
                                start=(r == 0), stop=(r == NC - 1))
                    for st4 in range(4):
                        st = sb_i * 4 + st4
                        ob = sb3.tile([128, CW], f32, tag="ob", bufs=4,
                                      name=f"ob_{st}")
                        nc.vector.tensor_add(ob[:], pos[st4][:], accs[st][:])
                        nc.sync.dma_start(
                            out[st * 128:(st + 1) * 128, :], ob[:])
            wdp_cm.__exit__(None, None, None)

    nc.compile()
    return nc


def prep_inputs(hidden_states, residual, alibi, attention_mask,
                qkv_w, qkv_b, dense_w, dense_b):
    """Full inputs -> list of 8 per-core input maps."""
    del attention_mask  # deterministic causal mask is baked into the kernel
    X = np.asarray(hidden_states, np.float32).reshape(S, H)
    XTfull = np.ascontiguousarray(X.T).astype(np_bf16)      # [H, S]
    res = np.asarray(residual, np.float32).reshape(S, H)
    alibi = np.asarray(alibi, np.float32).reshape(NH, S)    # slopes*pos
    qkv_w = np.asarray(qkv_w, np.float32).reshape(NH, 3, HD, H)
    qkv_b = np.asarray(qkv_b, np.float32).reshape(NH, 3, HD)
    dense_w = np.asarray(dense_w, np.float32)               # [H, H]
    dense_b = np.asarray(dense_b, np.float32)

    # diag masks by r = kt - 4*qc : allow kp <= qf - 128*r
    kp = np.arange(128)[:, None]
    qf = np.arange(512)[None, :]
    m4 = np.stack([np.where(kp <= qf - 128 * r, 0.0, MASK_VAL)
                   for r in range(4)], 0).astype(np.float32)       # [4,128,512]
    masks = np.ascontiguousarray(
        m4.transpose(1, 0, 2).reshape(128, 4 * 512))

    in_maps = []
    for c in range(NC):
        hs = list(range(c * HL, (c + 1) * HL))
        WqT = np.ascontiguousarray(
            (qkv_w[hs, 0] * INV_NORM).reshape(CW, H).T).astype(np_bf16)
        WkT = np.ascontiguousarray(qkv_w[hs, 1].reshape(CW, H).T).astype(np_bf16)
        WvT = np.ascontiguousarray(qkv_w[hs, 2].reshape(CW, H).T).astype(np_bf16)
        bq = (qkv_b[hs, 0] * INV_NORM).reshape(HL, HD)      # [4,128]
        bk = qkv_b[hs, 1].reshape(HL, HD)
        qb_cols = np.ascontiguousarray(
            np.concatenate([bq, bk], 0).T)                  # [128, 8]
        al = alibi[hs]                                      # [4, S]
        nalb_a = np.ascontiguousarray(np.tile(
            (-al).reshape(1, HL * S).astype(np.float32), (128, 1)))
        acol = np.ascontiguousarray(
            al.reshape(HL, ST, 128).transpose(2, 0, 1).reshape(128, HL * ST))
        cols = slice(c * CW, (c + 1) * CW)
        WdT = np.ascontiguousarray(dense_w[cols, :].T).astype(np_bf16)
        bv_full = qkv_b[:, 2, :].reshape(H).astype(np.float64)
        bv_dense = (bv_full @ dense_w[cols, :].T.astype(np.float64)
                    ).astype(np.float32)                # [CW]
        rescomb = np.ascontiguousarray(
            res[:, cols] + dense_b[None, cols] + bv_dense[None, :])
        in_maps.append({
            "XT": XTfull, "WqT": WqT, "WkT": WkT, "WvT": WvT,
            "qb_cols": qb_cols,
            "nalb": nalb_a, "alibi_cols": acol, "masks": masks,
            "ones128": np.ones((128, 128), dtype=np_bf16),
            "WdT": WdT, "rescomb": rescomb,
        })
    return in_maps


def assemble(results):
    return np.concatenate([r["out"] for r in results], axis=1).reshape(1, S, H)


# ---------------------------------------------------------------------------
# Harness entry point
# ---------------------------------------------------------------------------
from concourse.bass_utils import run_bass_kernel_spmd

_NC_CACHE = {}


def _get_nc():
    if "nc" not in _NC_CACHE:
        _NC_CACHE["nc"] = build_nc()
    return _NC_CACHE["nc"]


def kernel(**inputs):
    """Full (unsharded) Bloom-attention block on 8 NeuronCores.

    Shards tensor-parallel over heads (4 heads/core): per-core QKV
    projection + causal alibi attention, AllGather of the context, and a
    column-sharded dense projection with residual. Returns [1, 2048, 4096]
    float32.
    """
    nc = _get_nc()
    in_maps = prep_inputs(**inputs)
    res = run_bass_kernel_spmd(nc, in_maps, core_ids=list(range(NC)))
    return assemble(res.results).astype(np.float32)


def _kernel_profiled(**inputs):
    """kernel() + NTFF profiling; returns (output, hw_exec_time_ns)."""
    import sys as _sys
    import types as _types
    import concourse.bass_utils as _bu
    _bu.upload_artifacts = lambda tmpdir: "local://" + tmpdir
    if "antenv.axon_hooks" not in _sys.modules:
        try:
            from trn_agent_boot.trn_boot import _ntff_profile_via_ctypes
            _hook = _ntff_profile_via_ctypes("/opt/axon/libaxon_pjrt.so")
            _mod = _types.ModuleType("antenv.axon_hooks")
            _mod.get_axon_ntff_profile_hook = lambda: _hook
            _mod.set_axon_ntff_profile_hook = lambda h: None
            _sys.modules["antenv.axon_hooks"] = _mod
        except Exception:
            pass
    nc = _get_nc()
    in_maps = prep_inputs(**inputs)
    res = run_bass_kernel_spmd(nc, in_maps, core_ids=list(range(NC)),
                               trace=True)
    return assemble(res.results).astype(np.float32), res.exec_time_ns
